# revision 34
# baseline (speedup 1.0000x reference)
"""MoE (8 experts, top-2) Trainium2 kernel — FFN-sliced expert parallel, bf16.

Strategy:
  - Host: router (softmax + top-2 + renorm), dispatch tokens per expert.
  - Work unit = (expert, token-half, F-quarter): each of the 8 slots holds
    one expert; cores 0-3 process the first half of that expert's tokens
    (quarters 0-3 of D_FF), cores 4-7 the second half. Per-slot capacity
    = ceil(count_e / 2), so sum(caps) ~ 8196 token-slots/core vs the 8288
    of pair-max packing (PE floor = sum(caps) * 128 columns).
  - All matmuls bf16 (fp32 PSUM accumulate): measured end-to-end rel err
    ~3.8e-3. bf16 also enables Fast Weight Load (fp32 weights disable FWL).
  - Host: sum the 4 F-quarter partials (f32), add b2, weighted combine.

Device loop per slot (cap tokens, token groups of 512, passes of 2 groups):
  mm1: ps1[f128, g] = sum_dt W1t[dt, ft].T @ Xt[dt, g]   (bf16)
  h[ft, g] = Gelu(ps1 + b1[ft])  -> SBUF bf16
  mm2: ps2[d128, g] = sum_ft W2t[ft, dt2].T @ h[ft, g]   (bf16)
  y[dt2, g] = copy(ps2) -> DRAM bf16 (partials; summed f32 on host)

Single-shot latency tuning (global HWDGE ~625ns/DMA + serialized DMA
engines in the cost model; same ordering logic helps real hw):
  - slot 0's x arrives as per-(pass, dt) chunks, critical-first, and its
    mm1 runs pass-major so compute overlaps the remaining chunk DMAs.
  - the last slot writes y back per pass, group-major, with copies
    alternating between DVE and Activation, so only one small copy+DMA
    chain trails the final matmul.
"""

import numpy as np
import ml_dtypes

import concourse.bacc as bacc
import concourse.mybir as mybir
import concourse.tile as tile
from concourse.bass import ds, ts
from concourse.bass_utils import run_bass_kernel_spmd

P = 128
D_MODEL = 1024
D_FF = 4096
NUM_EXPERTS = 8
TOP_K = 2
NDT = D_MODEL // P   # 8 d-tiles
QF = D_FF // 4       # 1024: F-quarter width
NFQ = QF // P        # 8 f-tiles per quarter
GS = 512             # token group (matmul moving dim)
PASS_G = 2           # groups per psum pass (double-buffered)

f32 = mybir.dt.float32
bf16 = mybir.dt.bfloat16
BF16_NP = ml_dtypes.bfloat16
Y_DT = bf16  # y-partial output dtype (bf16 halves output DMA; rel err 3.8e-3 vs 3.4e-3)

_BUILT = {}


def _groups(cap):
    # evenly-sized groups (all within 2 tokens of each other): degenerate
    # tiny trailing groups would be LDWEIGHTS-bound on real hardware
    cap = int(cap)
    n = -(-cap // GS)
    base, extra = divmod(cap, n)
    sizes = [base + 1 if i < extra else base for i in range(n)]
    gs, o = [], 0
    for sz in sizes:
        gs.append((o, sz))
        o += sz
    return gs


def _passes(cap):
    gs = _groups(cap)
    return [gs[i : i + PASS_G] for i in range(0, len(gs), PASS_G)]


def _build(caps: tuple, repeats: int = 1):
    """Per-core module: NSLOTS independent quarter-FFN units, caps[j] tokens.

    DMA-order tuning (cost model: global HWDGE ~625ns/DMA + single
    DMA_ENGINES transfer device):
      - slot 0 x is loaded as per-(group, dt) chunk tiles, emitted
        critical-first (w1t0, then pass-0 chunks), so the first matmul
        starts at ~3us instead of ~19us.
      - b1 loads are emitted after slot 0's critical DMAs.
      - last slot's y is written back per pass-group so the final DMA
        after the last matmul is small.
    """
    nc = bacc.Bacc(None, target_bir_lowering=False)

    xt_d, w1_d, w2_d, b1_d, y_d = [], [], [], [], []
    for j, cap in enumerate(caps):
        xt_d.append(
            nc.declare_dram_parameter(f"xt{j}", [P, NDT, cap], bf16, isOutput=False)
        )
        # w1 tile k = ft*NDT + dt ; w2 tile k = dt2*NFQ + ft
        w1_d.append(
            nc.declare_dram_parameter(f"w1_{j}", [P, NFQ, NDT, P], bf16, isOutput=False)
        )
        w2_d.append(
            nc.declare_dram_parameter(f"w2_{j}", [P, NDT, NFQ, P], bf16, isOutput=False)
        )
        b1_d.append(
            nc.declare_dram_parameter(f"b1_{j}", [P, NFQ], f32, isOutput=False)
        )
        y_d.append(
            nc.declare_dram_parameter(f"y{j}", [P, NDT, cap], Y_DT, isOutput=True)
        )

    with tile.TileContext(nc) as tc:
        with (
            tc.tile_pool(name="const", bufs=1) as const_pool,
            tc.tile_pool(name="xt", bufs=2) as xt_pool,
            tc.tile_pool(name="w1", bufs=2) as w1_pool,
            tc.tile_pool(name="w2", bufs=1) as w2_pool,
            tc.tile_pool(name="h", bufs=1) as h_pool,
            tc.tile_pool(name="yo", bufs=3) as y_pool,
            tc.tile_pool(name="ps1", bufs=2, space="PSUM") as ps1_pool,
            tc.tile_pool(name="ps2", bufs=2, space="PSUM") as ps2_pool,
        ):
            nslots = len(caps)
            b1_sb = [None] * nslots

            def load_b1(j):
                if b1_sb[j] is None:
                    t = const_pool.tile([P, NFQ], f32, name=f"b1sb{j}")
                    nc.sync.dma_start(out=t[:], in_=b1_d[j][:])
                    b1_sb[j] = t

            for it, j in enumerate(jj % nslots for jj in range(nslots * repeats)):
                cap = caps[j]
                first = it == 0
                last_slot = it == nslots * repeats - 1
                passes = _passes(cap)
                groups = _groups(cap)

                w1_sb = [None] * NFQ

                def load_w1(ft, j=j):
                    if w1_sb[ft] is None:
                        t = w1_pool.tile([P, NDT, P], bf16, name=f"w1t{ft}")
                        nc.sync.dma_start(
                            out=t[:], in_=w1_d[j][:, ts(ft, 1)].squeeze()
                        )
                        w1_sb[ft] = t

                if first:
                    # x as per-(pass, dt) chunk tiles, emitted critical-first
                    # and interleaved with the w1 tiles in the order slot 0's
                    # pass-major mm1 consumes them (w1t_k needed one ft-row
                    # ahead; c1/c2 only at pass 1/2)
                    pranges = []
                    for pgs in passes:
                        plo = pgs[0][0]
                        pranges.append((plo, pgs[-1][0] + pgs[-1][1] - plo))
                    x_ck = [[None] * len(pranges) for _ in range(NDT)]

                    def load_x(dt, ci, j=j):
                        co, csz = pranges[ci]
                        t = xt_pool.tile(
                            [P, csz], bf16, name=f"x{dt}c{ci}", bufs=1
                        )
                        nc.sync.dma_start(
                            out=t[:],
                            in_=xt_d[j][:, ts(dt, 1)].squeeze()[:, ds(co, csz)],
                        )
                        x_ck[dt][ci] = t

                    load_w1(0)
                    for dt in range(4):
                        load_x(dt, 0)
                    load_w1(1)
                    for dt in range(4, NDT):
                        load_x(dt, 0)
                    load_b1(j)
                    for ft in range(2, NFQ):
                        load_w1(ft)
                    for ci in range(1, len(pranges)):
                        for dt in range(NDT):
                            load_x(dt, ci)

                    def x_op(dt, gi, go, gsz):
                        ci = gi // PASS_G
                        return x_ck[dt][ci][:, ds(go - pranges[ci][0], gsz)]
                else:
                    load_b1(j)
                    x_sb = []
                    for dt in range(NDT):
                        t = xt_pool.tile([P, cap], bf16, name=f"x{dt}")
                        nc.sync.dma_start(
                            out=t[:], in_=xt_d[j][:, ts(dt, 1)].squeeze()
                        )
                        x_sb.append(t)

                    def x_op(dt, gi, go, gsz):
                        return x_sb[dt][:, ds(go, gsz)]

                for ft in range(NFQ):
                    load_w1(ft)
                w2_sb = []
                for dt2 in range(NDT):
                    t = w2_pool.tile([P, NFQ, P], bf16, name=f"w2t{dt2}")
                    nc.sync.dma_start(out=t[:], in_=w2_d[j][:, ts(dt2, 1)].squeeze())
                    w2_sb.append(t)

                # ---- mm1 + gelu -> h (bf16, per-ft tiles) ----
                # slot 0 runs pass-major (all ft on pass p before pass p+1)
                # so compute for pass 0 overlaps the remaining x-chunk DMAs;
                # later slots run ft-major (x fully prefetched).
                h_sb = [
                    h_pool.tile([P, cap], bf16, name=f"h{ft}")
                    for ft in range(NFQ)
                ]

                def mm1_pass(ft, pi, pgs):
                    gi0 = pi * PASS_G
                    ps = [
                        ps1_pool.tile([P, g[1]], f32, name=f"ps1_{i}")
                        for i, g in enumerate(pgs)
                    ]
                    for dt in range(NDT):
                        for i, (go, gsz) in enumerate(pgs):
                            nc.tensor.matmul(
                                ps[i][:],
                                w1_sb[ft][:, ts(dt, 1)].squeeze(),
                                x_op(dt, gi0 + i, go, gsz),
                                start=(dt == 0),
                                stop=(dt == NDT - 1),
                                skip_group_check=True,
                            )
                    for i, (go, gsz) in enumerate(pgs):
                        nc.scalar.activation(
                            h_sb[ft][:, ds(go, gsz)],
                            ps[i][:],
                            mybir.ActivationFunctionType.Gelu,
                            bias=b1_sb[j][:, ts(ft, 1)],
                        )

                if first:
                    for pi, pgs in enumerate(passes):
                        for ft in range(NFQ):
                            mm1_pass(ft, pi, pgs)
                else:
                    for ft in range(NFQ):
                        for pi, pgs in enumerate(passes):
                            mm1_pass(ft, pi, pgs)

                # ---- mm2 -> y ----
                def mm2_pass(dt2, pgs, y_ap, y_ap_off, ps_par=0):
                    ps = [
                        ps2_pool.tile(
                            [P, g[1]], f32, name=f"ps2_{(i + ps_par) % 2}"
                        )
                        for i, g in enumerate(pgs)
                    ]
                    if last_slot:
                        # group-major: group i's copy drains during group
                        # i+1's matmuls, shrinking the post-last-matmul tail
                        order = [
                            (ft, i) for i in range(len(pgs)) for ft in range(NFQ)
                        ]
                    else:
                        order = [
                            (ft, i) for ft in range(NFQ) for i in range(len(pgs))
                        ]
                    for ft, i in order:
                        go, gsz = pgs[i]
                        nc.tensor.matmul(
                            ps[i][:],
                            w2_sb[dt2][:, ts(ft, 1)].squeeze(),
                            h_sb[ft][:, ds(go, gsz)],
                            start=(ft == 0),
                            stop=(ft == NFQ - 1),
                            skip_group_check=True,
                        )
                    for i, (go, gsz) in enumerate(pgs):
                        dst = y_ap[:, ds(go - y_ap_off, gsz)]
                        if last_slot and ((dt2 + i) & 1):
                            # spread tail copies across DVE and the idle
                            # Activation engine so they drain in parallel
                            nc.scalar.activation(
                                dst, ps[i][:], mybir.ActivationFunctionType.Copy
                            )
                        else:
                            nc.vector.tensor_copy(dst, ps[i][:])

                for dt2 in range(NDT):
                    y_sb = y_pool.tile([P, cap], Y_DT, name="ysb", tag="ysb")
                    if last_slot:
                        # per-group writeback: group-major mm2 lets group
                        # g's copy+DMA drain during group g+1's matmuls, so
                        # only one small copy+DMA trails the final matmul
                        for pgs in passes:
                            mm2_pass(dt2, pgs, y_sb, 0)
                            for go, gsz in pgs:
                                nc.sync.dma_start(
                                    out=y_d[j][:, ts(dt2, 1)].squeeze()[
                                        :, ds(go, gsz)
                                    ],
                                    in_=y_sb[:, ds(go, gsz)],
                                )
                    else:
                        for pgs in passes:
                            mm2_pass(dt2, pgs, y_sb, 0)
                        nc.sync.dma_start(
                            out=y_d[j][:, ts(dt2, 1)].squeeze(), in_=y_sb[:]
                        )

    nc.compile()
    return nc


def _get_built(caps, repeats: int = 1):
    key = (tuple(caps), repeats)
    if key not in _BUILT:
        _BUILT[key] = _build(tuple(caps), repeats)
    return _BUILT[key]


def _route(x_flat, Wr, br):
    """Router: softmax over experts, top-2, renormalized. Pure numpy."""
    logits = x_flat.astype(np.float32) @ Wr.astype(np.float32) + br.astype(np.float32)
    m = logits.max(axis=-1, keepdims=True)
    p = np.exp(logits - m)
    p /= p.sum(axis=-1, keepdims=True)
    i0 = np.argmax(p, axis=-1)
    pm = p.copy()
    pm[np.arange(p.shape[0]), i0] = -np.inf
    i1 = np.argmax(pm, axis=-1)
    w0 = p[np.arange(p.shape[0]), i0]
    w1 = p[np.arange(p.shape[0]), i1]
    s = w0 + w1
    return i0, i1, w0 / s, w1 / s


def _pad8(n):
    # bf16 tiles only need even (4-byte-aligned) token counts
    return max(GS // 4, (n + 1) // 2 * 2)


def kernel(x, Wr, br, W1, b1, W2, b2, _run_kwargs=None):
    x = np.asarray(x)
    B, L, D = x.shape
    T = B * L
    x_flat = np.ascontiguousarray(x.reshape(T, D), dtype=np.float32)

    i0, i1, w0, w1c = _route(x_flat, Wr, br)

    rows_l, wts_l = [], []
    for e in range(NUM_EXPERTS):
        sel = (i0 == e) | (i1 == e)
        rows = np.nonzero(sel)[0]
        w = np.where(i0[rows] == e, w0[rows], w1c[rows]).astype(np.float32)
        rows_l.append(rows)
        wts_l.append(w)

    counts = np.array([len(r) for r in rows_l])
    order = np.argsort(-counts, kind="stable")
    # unit u = expert order[u], token-halved: cores 0-3 (quarters 0-3) run
    # the first half of each expert's tokens, cores 4-7 the second half
    units = [int(e) for e in order]
    mids = [int(counts[e] + 1) // 2 for e in units]
    caps = tuple(_pad8(m) for m in mids)
    nc = _get_built(caps)

    # packed x per (unit, half) and weights per (expert, quarter)
    xt_uh, w1_e, w2_e, b1_e = {}, {}, {}, {}
    for u, e in enumerate(units):
        cap = int(caps[u])
        rows = rows_l[e]
        for half in range(2):
            sl = rows[: mids[u]] if half == 0 else rows[mids[u] :]
            xe = np.zeros((cap, D_MODEL), dtype=np.float32)
            xe[: len(sl)] = x_flat[sl]
            # [cap, D] -> [D, cap] -> [NDT, P, cap] -> [P, NDT, cap]
            xt_uh[u, half] = np.ascontiguousarray(
                xe.T.reshape(NDT, P, cap).transpose(1, 0, 2)
            ).astype(BF16_NP)
        w1f = np.asarray(W1[e], dtype=np.float32)  # [D, F]
        w2f = np.asarray(W2[e], dtype=np.float32)  # [F, D]
        b1f = np.asarray(b1[e], dtype=np.float32)  # [F]
        w1_e[e], w2_e[e], b1_e[e] = [], [], []
        for q in range(4):
            w1q = w1f[:, q * QF : (q + 1) * QF]  # [1024, 1024]
            # [NDT, P, NFQ, P] -> [P, NFQ, NDT, P]
            w1_e[e].append(
                np.ascontiguousarray(
                    w1q.reshape(NDT, P, NFQ, P).transpose(1, 2, 0, 3)
                ).astype(BF16_NP)
            )
            w2q = w2f[q * QF : (q + 1) * QF, :]  # [1024, 1024]
            # [NFQ, P, NDT, P] -> [P, NDT, NFQ, P]
            w2_e[e].append(
                np.ascontiguousarray(
                    w2q.reshape(NFQ, P, NDT, P).transpose(1, 2, 0, 3)
                ).astype(BF16_NP)
            )
            b1_e[e].append(
                np.ascontiguousarray(
                    b1f[q * QF : (q + 1) * QF].reshape(NFQ, P).T
                )
            )

    in_maps = []
    for c in range(NUM_EXPERTS):
        q, half = c % 4, c // 4
        m = {}
        for u, e in enumerate(units):
            m[f"xt{u}"] = xt_uh[u, half]
            m[f"w1_{u}"] = w1_e[e][q]
            m[f"w2_{u}"] = w2_e[e][q]
            m[f"b1_{u}"] = b1_e[e][q]
        in_maps.append(m)

    kw = dict(_run_kwargs or {})
    res = run_bass_kernel_spmd(nc, in_maps, list(range(NUM_EXPERTS)), **kw)

    # Combine: per (unit, half) sum the 4 quarter-partials, add b2,
    # weighted scatter into the output rows of that token half
    out = np.zeros((T, D_MODEL), dtype=np.float32)
    for u, e in enumerate(units):
        cap = int(caps[u])
        rows = rows_l[e]
        b2e = np.asarray(b2[e], dtype=np.float32)
        for half in range(2):
            sl = rows[: mids[u]] if half == 0 else rows[mids[u] :]
            wts = (
                wts_l[e][: mids[u]] if half == 0 else wts_l[e][mids[u] :]
            )
            acc = None
            for q in range(4):
                part = np.asarray(
                    res.results[q + 4 * half][f"y{u}"], dtype=np.float32
                )
                acc = part if acc is None else acc + part
            ye = acc.transpose(1, 0, 2).reshape(D_MODEL, cap)  # [D, cap]
            ye = ye[:, : len(sl)].T + b2e
            out[sl] += wts[:, None] * ye

    kernel._last_result = res
    kernel._last_in_maps = in_maps
    kernel._last_cap = caps
    return out.reshape(B, L, D_MODEL)


def make_bench_runner(nc, in_maps, n_cores=NUM_EXPERTS):
    """Device-resident repeat-execution runner for timing (mirrors
    bass2jax.run_bass_via_pjrt's multi-core path, but stages inputs on
    device once and creates donated zero outputs on-device)."""
    import jax
    import jax.numpy as jnp
    from jax.experimental.shard_map import shard_map
    from jax.sharding import Mesh, NamedSharding, PartitionSpec

    from concourse import bass2jax
    from concourse import mybir as _mybir

    bass2jax.install_neuronx_cc_hook()

    part_name = (
        nc.partition_id_tensor.name if nc.partition_id_tensor else None
    )
    in_names, out_names, out_avals = [], [], []
    for alloc in nc.m.functions[0].allocations:
        if not isinstance(alloc, _mybir.MemoryLocationSet):
            continue
        name = alloc.memorylocations[0].name
        if alloc.kind == "ExternalInput":
            if name != part_name:
                in_names.append(name)
        elif alloc.kind == "ExternalOutput":
            out_names.append(name)
            out_avals.append(
                jax.core.ShapedArray(
                    tuple(alloc.tensor_shape), _mybir.dt.np(alloc.dtype)
                )
            )
    n_params = len(in_names)
    all_in = in_names + out_names
    if part_name is not None:
        all_in = all_in + [part_name]

    def _body(*args):
        operands = list(args)
        if part_name is not None:
            operands.append(bass2jax.partition_id_tensor())
        outs = bass2jax._bass_exec_p.bind(
            *operands,
            out_avals=tuple(out_avals),
            in_names=tuple(all_in),
            out_names=tuple(out_names),
            lowering_input_output_aliases=(),
            sim_require_finite=True,
            sim_require_nnan=True,
            nc=nc,
        )
        return tuple(outs)

    devices = jax.devices()[:n_cores]
    mesh = Mesh(np.asarray(devices), ("core",))
    spec = NamedSharding(mesh, PartitionSpec("core"))
    donate = tuple(range(n_params, n_params + len(out_names)))
    sharded = jax.jit(
        shard_map(
            _body,
            mesh=mesh,
            in_specs=(PartitionSpec("core"),) * (n_params + len(out_names)),
            out_specs=(PartitionSpec("core"),) * len(out_names),
            check_rep=False,
        ),
        donate_argnums=donate,
        keep_unused=True,
    )
    din = [
        jax.device_put(
            np.concatenate([m[name] for m in in_maps], axis=0), spec
        )
        for name in in_names
    ]
    zero_shapes = [
        (n_cores * a.shape[0], *a.shape[1:]) for a in out_avals
    ]
    zeros_fn = jax.jit(
        lambda: tuple(
            jnp.zeros(s, a.dtype) for s, a in zip(zero_shapes, out_avals)
        ),
        out_shardings=tuple(spec for _ in out_avals),
    )

    def run_once():
        return sharded(*din, *zeros_fn())

    def zeros_only():
        return zeros_fn()

    return run_once, zeros_only

